# revision 1
# baseline (speedup 1.0000x reference)
"""Trainium2 Bass kernel for nn_DAGModel_88630945120510 (gnn_message_passing).

Data-parallel over batch: 32 batches -> 8 cores x 4 batches. The node buffer
lives in DRAM as [16386 rows, 4 batch, 128] fp32 so one row (2048B) holds all
4 local batches of one node vector -> single gather descriptor at DMA line
rate. Per depth step: dma_gather parent rows (host-precomputed p-major index
lists) -> DVE sum over parent slots (f32) -> PE transpose to feature-major
(f32 -> PSUM, cast bf16 on the ACT copy-out) -> bf16 MLP (W1/W2 weights in
bf16, f32 PSUM accumulate) with ACT relu/bias -> f32 residual add in
token-major -> contiguous DMA write of the new 512 rows. bf16 is used only on
the MLP input/weight path (one rounding per step, no storage compounding);
the node-vector recurrence itself stays f32.

Host-side (allowed: plain input marshalling): indices are remapped/sorted per
step (nodes sorted desc by #nonzero parents; parents compacted) so the gather
only reads the non-padding parents (~29% fewer bytes). Node order inside each
depth block is permuted; emb_table columns are pre-permuted to match and the
final output is un-permuted on the host.
"""

import hashlib
import numpy as np
from contextlib import ExitStack

import concourse.bass as bass
import concourse.mybir as mybir
import concourse.tile as tile
from concourse import bacc
from concourse.bass_utils import run_bass_kernel_spmd
from concourse._compat import cdiv

F32 = mybir.dt.float32
F32R = mybir.dt.float32r
BF16 = mybir.dt.bfloat16
I16 = mybir.dt.int16

B, H, E = 32, 128, 64
D, N, P = 32, 512, 8
TOTAL = 2 + D * N
NCORES = 8
BL = B // NCORES          # batches per core
ROW = BL * H              # fp32 elems per DRAM row record
CH = 7                    # G-tile chunks; 896 idx/gather (4 preps/step < ring)
SORT = True               # sorted-prefix gather (skip zero parents)
SCRATCH = 65536           # SWDGE ring bytes/partition (4096 desc slots)
SPLIT_MIN = 1             # first step handled with the stale/fresh split


# ----------------------------------------------------------------------------
# host-side layout builder
# ----------------------------------------------------------------------------

def build_layout(parent_idx, sort=SORT, nsteps=D):
    """Per-step gather index lists + psum-op metadata + permutations.

    For steps d >= SPLIT_MIN the gather lists contain only STALE parents
    (storage rows < bound_{d-1}); fresh parents (the previous depth's nodes)
    are applied on-device as 16 PE matmuls against a per-step 0/1 selection
    matrix A (columns = this step's storage slots, rows = previous depth's
    local rows) times the previous step's vt tile, which is still in SBUF.
    This removes the wb_{d-1} -> gather_d dependency, so each step's gather
    (Q7 desc-gen + DMA) runs during the PREVIOUS step's compute."""
    parent_idx = np.asarray(parent_idx)
    assert parent_idx.shape == (D, N, P)
    pos = np.zeros((D, N), np.int64)   # pos[d][n] = storage slot of node n
    sig = np.zeros((D, N), np.int64)   # sig[d][j] = node stored at slot j
    rowmap = np.zeros(TOTAL, np.int64)
    rowmap[0], rowmap[1] = 0, 1
    steps = []
    packed_cols = []
    afix_parts = []
    col_off = 0
    for d in range(nsteps):
        bound = 2 + d * N
        pv = parent_idx[d].astype(np.int64).copy()   # [N, P]
        pv[(pv < 0) | (pv >= bound)] = 0             # pad/OOB -> zero row
        split = d >= SPLIT_MIN
        if split:
            prev_lo = 2 + (d - 1) * N
            fresh = pv >= prev_lo                    # previous depth's rows
            pv_stale = pv.copy()
            pv_stale[fresh] = 0
        else:
            fresh = None
            pv_stale = pv
        if sort:
            k = (pv_stale != 0).sum(1)
            order = np.argsort(-k, kind="stable")
        else:
            order = np.arange(N)
        sig[d] = order
        pos[d, order] = np.arange(N)
        rowmap[bound : bound + N] = bound + pos[d]
        pvs = pv_stale[order]                        # [N, P] node-sorted
        if sort:
            ks = (pvs != 0).sum(1)
            comp = np.zeros_like(pvs)
            for j in range(N):
                nz = pvs[j][pvs[j] != 0]
                comp[j, : len(nz)] = nz
            pvs = comp
            cps = [int((ks > p).sum()) for p in range(P)]
        else:
            cps = [N] * P

        if split:
            # A[kc*4+mc][q%128, s%128]: fresh-parent multiplicity of node at
            # slot s for previous-depth local row q (q = rowmap - prev_lo)
            A = np.zeros((16, 128, 128), np.float32)
            nn_, pp_ = np.nonzero(fresh)
            for n0, p0 in zip(nn_, pp_):
                q = rowmap[pv[n0, p0]] - prev_lo
                s = pos[d][n0]
                A[(q // 128) * 4 + (s // 128), q % 128, s % 128] += 1.0
            afix_parts.append(
                np.ascontiguousarray(A.transpose(1, 0, 2).reshape(128, 2048)))

        segs = []          # (p, global col offset, padded len)
        idx_parts = []
        off = 0
        for p in range(P):
            cp = cps[p]
            cpp = N if p == 0 else (cdiv(cp, 128) * 128 if cp > 0 else 0)
            if cpp == 0:
                continue
            col = np.zeros(cpp, np.int64)
            m = min(cp, cpp)
            col[:m] = rowmap[pvs[:m, p]]
            idx_parts.append(col)
            segs.append((p, off, cpp))
            off += cpp
        L = off
        idx_list = np.concatenate(idx_parts)
        assert idx_list.shape == (L,) and L % 128 == 0
        assert idx_list.max() < 2 ** 15

        # psum ops: slot segment -> per-G-tile runs (G tiles are CH chunks)
        nch = L // 128
        ntiles = cdiv(nch, CH)
        ops = []   # (is_copy, dst_chunk, nchunks, tile, local_chunk)
        for (p, soff, cpp) in segs:
            gc0 = soff // 128
            ncht = cpp // 128
            c = 0
            while c < ncht:
                t = (gc0 + c) // CH
                lc = (gc0 + c) % CH
                run = min(CH - lc, ncht - c)
                ops.append((p == 0, c, run, t, lc))
                c += run

        # pack idx list: linear i -> partition i%16, col i//16; replicate x8
        cols = L // 16
        pk = idx_list.astype(np.int16).reshape(cols, 16).T
        packed_cols.append(np.tile(pk, (8, 1)))
        steps.append(dict(L=L, nch=nch, ntiles=ntiles, ops=ops,
                          col_off=col_off, cols=cols, bound=bound,
                          split=split, fix=len(afix_parts) - 1 if split else -1))
        col_off += cols

    idx_packed = np.concatenate(packed_cols, axis=1)   # [128, col_off]
    afix = (np.stack(afix_parts) if afix_parts
            else np.zeros((1, 128, 2048), np.float32))  # [NFIX, 128, 2048]
    return dict(steps=steps, idx_packed=idx_packed, pos=pos, sig=sig,
                total_cols=col_off, afix=afix)


# ----------------------------------------------------------------------------
# device kernel
# ----------------------------------------------------------------------------

def build_nc(meta, repeat=1):
    nc = bacc.Bacc("TRN2", target_bir_lowering=False, debug=False,
                   dynamic_dma_scratch_size=SCRATCH)

    bufrows = nc.declare_dram_parameter("bufrows", [TOTAL, BL, H], F32,
                                        isOutput=True)
    init2 = nc.declare_dram_parameter("init2", [2, BL, H], F32, isOutput=False)
    embt = nc.declare_dram_parameter("embt", [E, D * N], BF16, isOutput=False)
    w1pv_d = nc.declare_dram_parameter("w1pv", [H, H], BF16, isOutput=False)
    w1ne_d = nc.declare_dram_parameter("w1ne", [E, H], BF16, isOutput=False)
    w2t_d = nc.declare_dram_parameter("w2t", [H, H], BF16, isOutput=False)
    b1_d = nc.declare_dram_parameter("b1", [H, 1], F32, isOutput=False)
    b2_d = nc.declare_dram_parameter("b2", [H, 1], F32, isOutput=False)
    ident_d = nc.declare_dram_parameter("ident", [H, H], F32, isOutput=False)
    idxs_d = nc.declare_dram_parameter("idxs", [128, meta["total_cols"]], I16,
                                       isOutput=False)
    nfix = meta["afix"].shape[0]
    afix_d = nc.declare_dram_parameter("afix", [nfix, 128, 2048], BF16,
                                       isOutput=False)

    steps = meta["steps"]

    with tile.TileContext(nc) as tc, ExitStack() as ctx:
        const = ctx.enter_context(tc.tile_pool(name="const", bufs=1))
        gpool = ctx.enter_context(tc.tile_pool(name="g", bufs=2))
        apool = ctx.enter_context(tc.tile_pool(name="acc", bufs=2))
        spool = ctx.enter_context(tc.tile_pool(name="s", bufs=4))
        nepool = ctx.enter_context(tc.tile_pool(name="ne", bufs=2))
        vpool = ctx.enter_context(tc.tile_pool(name="v", bufs=2))
        afpool = ctx.enter_context(tc.tile_pool(name="af", bufs=2))
        vbpool = ctx.enter_context(tc.tile_pool(name="vb", bufs=2))
        psum = ctx.enter_context(tc.tile_pool(name="ps", bufs=4, space="PSUM"))

        # constants
        idxs_sb = const.tile([128, meta["total_cols"]], I16, tag="idxs")
        nc.sync.dma_start(idxs_sb[:], idxs_d[:])
        w1pv = const.tile([H, H], BF16, tag="w1pv")
        nc.sync.dma_start(w1pv[:], w1pv_d[:])
        w1ne = const.tile([E, H], BF16, tag="w1ne")
        nc.sync.dma_start(w1ne[:], w1ne_d[:])
        w2t = const.tile([H, H], BF16, tag="w2t")
        nc.sync.dma_start(w2t[:], w2t_d[:])
        b1 = const.tile([H, 1], F32, tag="b1")
        nc.sync.dma_start(b1[:], b1_d[:])
        b2 = const.tile([H, 1], F32, tag="b2")
        nc.sync.dma_start(b2[:], b2_d[:])
        identf = const.tile([H, H], F32, tag="ident")
        nc.sync.dma_start(identf[:], ident_d[:])

        # init rows 0 (zeros) and 1 (root embedding)
        nc.sync.dma_start(bufrows[0:2, :, :], init2[:])

        def emit_gathers(d):
            s = steps[d]
            # split steps read only rows < bound_{d-1}: no dependency on the
            # previous step's writeback, so the gather runs a step early.
            hi = s["bound"] - N if s["split"] else s["bound"]
            src = bufrows[0:hi, :, :].rearrange("r b h -> r (b h)")
            gts = []
            for t in range(s["ntiles"]):
                ncht = min(CH, s["nch"] - t * CH)
                Lt = ncht * 128
                g = gpool.tile([128, CH, BL, H], F32, tag=f"g{t % 2}")
                c0 = s["col_off"] + t * CH * 8
                nc.gpsimd.dma_gather(
                    g[:, 0:ncht, :, :].rearrange("p c b h -> p c (b h)"),
                    src, idxs_sb[:, c0:c0 + Lt // 16], Lt, Lt, ROW)
                gts.append(g)
            return gts

        def emit_steps():
            vtb_prev = None
            gts_next = None
            for d in range(len(steps)):
                s = steps[d]
                bound = s["bound"]

                gts = gts_next if gts_next is not None else emit_gathers(d)
                gts_next = None

                ne_t = nepool.tile([E, N], BF16, tag="ne")
                nc.sync.dma_start(ne_t[:], embt[:, d * N:(d + 1) * N])

                # P-sum of stale parents into acc [tok%128, nhi, b, f]
                acc = apool.tile([128, 4, BL, H], F32, tag="acc")
                for (is_copy, dc, ncg, t, lc) in s["ops"]:
                    dst = acc[:, dc:dc + ncg, :, :]
                    gsrc = gts[t][:, lc:lc + ncg, :, :]
                    if is_copy:
                        nc.vector.tensor_copy(dst, gsrc)
                    else:
                        nc.vector.tensor_add(dst, dst, gsrc)

                if s["split"]:
                    # fresh parents = previous depth's nodes, still in SBUF
                    # (vtb_prev): acc += A.T @ vtb via 16 accumulating MMs
                    af = afpool.tile([128, 16 * 128], BF16, tag="af")
                    nc.sync.dma_start(af[:], afix_d[s["fix"], :, :])
                    for mc in range(4):
                        pf = psum.tile([128, BL, H], F32, tag="pf")
                        pfv = pf[:].rearrange("p b h -> p (b h)")
                        for kc in range(4):
                            i = (kc * 4 + mc) * 128
                            nc.tensor.matmul(
                                pfv, af[:, i:i + 128],
                                vtb_prev[:, kc, :, :].rearrange(
                                    "p b h -> p (b h)"),
                                start=(kc == 0), stop=(kc == 3))
                        nc.vector.tensor_add(acc[:, mc, :, :],
                                             acc[:, mc, :, :], pf[:])

                # next step's stale gather: desc-gen + DMA overlap the MLP
                # below (no dep on this step's writeback).
                if d + 1 < len(steps) and steps[d + 1]["split"]:
                    gts_next = emit_gathers(d + 1)

                vt = vpool.tile([128, 4, BL, H], F32, tag="vt")
                for b in range(BL):
                    ps_tp = psum.tile([128, N], F32, tag="ps")
                    for nhi in range(4):
                        nc.tensor.transpose(ps_tp[:, nhi * 128:(nhi + 1) * 128],
                                            acc[:, nhi, b, :], identf[:])
                    pvT = spool.tile([128, N], BF16, tag="pvT")
                    nc.scalar.activation(pvT[:], ps_tp[:],
                                         mybir.ActivationFunctionType.Copy)
                    ph1 = psum.tile([128, N], F32, tag="ps")
                    nc.tensor.matmul(ph1[:], w1ne[:], ne_t[:], start=True,
                                     stop=False)
                    nc.tensor.matmul(ph1[:], w1pv[:], pvT[:], start=False,
                                     stop=True)
                    h1 = spool.tile([128, N], BF16, tag="h1")
                    nc.scalar.activation(h1[:], ph1[:],
                                         mybir.ActivationFunctionType.Relu,
                                         bias=b1[:])
                    ph2 = psum.tile([128, N], F32, tag="ps")
                    nc.tensor.matmul(ph2[:], w2t[:], h1[:], start=True, stop=True)
                    h2 = spool.tile([128, N], F32, tag="h2")
                    nc.scalar.activation(h2[:], ph2[:],
                                         mybir.ActivationFunctionType.Identity,
                                         bias=b2[:])
                    ps_ht = psum.tile([128, 4, H], F32, tag="ps")
                    for nhi in range(4):
                        nc.tensor.transpose(ps_ht[:, nhi, :],
                                            h2[:, nhi * 128:(nhi + 1) * 128],
                                            identf[:])
                    nc.vector.tensor_add(vt[:, :, b, :], acc[:, :, b, :],
                                         ps_ht[:])

                if d + 1 < len(steps) and steps[d + 1]["split"]:
                    vtb = vbpool.tile([128, 4, BL, H], BF16, tag="vtb")
                    nc.scalar.activation(vtb[:], vt[:],
                                         mybir.ActivationFunctionType.Copy)
                    vtb_prev = vtb

                rb = bound
                dst = bufrows[rb:rb + N, :, :].rearrange("(c p) b h -> p c b h",
                                                         p=128)
                nc.sync.dma_start(dst, vt[:])


        if repeat > 1:
            with tc.For_i(0, repeat, 1):
                emit_steps()
        else:
            emit_steps()

    nc.compile()
    return nc


# ----------------------------------------------------------------------------
# entry point
# ----------------------------------------------------------------------------

_CACHE = {}


def _get_compiled(parent_idx):
    key = hashlib.sha1(np.asarray(parent_idx).tobytes()).hexdigest()
    if key not in _CACHE:
        meta = build_layout(parent_idx)
        nc = build_nc(meta)
        _CACHE[key] = (nc, meta)
    return _CACHE[key]


def _prepare(inputs):
    embedding = np.asarray(inputs["embedding"], np.float32)
    emb_table = np.asarray(inputs["emb_table"], np.float32)
    W1 = np.asarray(inputs["W1"], np.float32)
    b1 = np.asarray(inputs["b1"], np.float32)
    W2 = np.asarray(inputs["W2"], np.float32)
    b2 = np.asarray(inputs["b2"], np.float32)
    parent_idx = np.asarray(inputs["parent_idx"])

    nc, meta = _get_compiled(parent_idx)
    sig = meta["sig"]

    import ml_dtypes
    bf16 = ml_dtypes.bfloat16
    embt = np.ascontiguousarray(
        np.concatenate([emb_table[2 + d * N + sig[d]] for d in range(D)],
                       axis=0).T)                        # [E, D*N]
    shared = dict(
        embt=embt.astype(bf16),
        w1pv=np.ascontiguousarray(W1[:, :H].T).astype(bf16),
        w1ne=np.ascontiguousarray(W1[:, H:].T).astype(bf16),
        w2t=np.ascontiguousarray(W2.T).astype(bf16),
        b1=b1.reshape(H, 1).copy(),
        b2=b2.reshape(H, 1).copy(),
        ident=np.eye(H, dtype=np.float32),
        idxs=meta["idx_packed"],
        afix=meta["afix"].astype(bf16),
    )
    in_maps = []
    for c in range(NCORES):
        init2 = np.zeros((2, BL, H), np.float32)
        init2[1] = embedding[c * BL:(c + 1) * BL]
        in_maps.append(dict(shared, init2=init2))
    return nc, meta, in_maps


def _run(inputs, trace=False):
    nc, meta, in_maps = _prepare(inputs)
    res = run_bass_kernel_spmd(nc, in_maps, list(range(NCORES)), trace=trace)

    # un-permute rows into the reference output layout
    R = np.empty(TOTAL - 1, np.int64)
    R[0] = 1
    pos = meta["pos"]
    for d in range(D):
        R[1 + d * N: 1 + (d + 1) * N] = 2 + d * N + pos[d]
    out = np.empty((B, TOTAL - 1, H), np.float32)
    for c in range(NCORES):
        br = res.results[c]["bufrows"]                  # [TOTAL, BL, H]
        out[c * BL:(c + 1) * BL] = br[R].transpose(1, 0, 2)
    return out, res


def kernel(**inputs) -> np.ndarray:
    out, _ = _run(inputs, trace=False)
    return out



# revision 35
# speedup vs baseline: 1.5994x; 1.5994x over previous
"""Trainium2 Bass kernel for nn_DAGModel_88630945120510 (gnn_message_passing).

Data-parallel over batch: 32 batches -> 8 cores x 4 batches. The node buffer
lives in DRAM as [16386 rows, 4 batch, 128] fp32 so one row (2048B) holds all
4 local batches of one node vector -> single gather descriptor at DMA line
rate. Per depth step: dma_gather parent rows (host-precomputed p-major index
lists) -> DVE sum over parent slots (f32) -> PE transpose to feature-major
(f32 -> PSUM, cast bf16 on the ACT copy-out) -> bf16 MLP (W1/W2 weights in
bf16, f32 PSUM accumulate) with ACT relu/bias -> f32 residual add in
token-major -> contiguous DMA write of the new 512 rows. bf16 is used only on
the MLP input/weight path (one rounding per step, no storage compounding);
the node-vector recurrence itself stays f32.

Host-side (allowed: plain input marshalling): indices are remapped/sorted per
step (nodes sorted desc by #nonzero parents; parents compacted) so the gather
only reads the non-padding parents (~29% fewer bytes). Node order inside each
depth block is permuted; emb_table columns are pre-permuted to match and the
final output is un-permuted on the host.
"""

import hashlib
import numpy as np
from contextlib import ExitStack

import concourse.bass as bass
import concourse.mybir as mybir
import concourse.tile as tile
from concourse import bacc
from concourse.bass_utils import run_bass_kernel_spmd
from concourse._compat import cdiv

F32 = mybir.dt.float32
F32R = mybir.dt.float32r
BF16 = mybir.dt.bfloat16
I16 = mybir.dt.int16

B, H, E = 32, 128, 64
D, N, P = 32, 512, 8
TOTAL = 2 + D * N
NCORES = 8
BL = B // NCORES          # batches per core
ROW = BL * H              # fp32 elems per DRAM row record
CH = 7                    # G-tile chunks; 896 idx/gather
SORT = True               # sorted-prefix gather (skip zero parents)
SCRATCH = 65536           # SWDGE ring bytes/partition (4096 desc slots)
SPLIT_MIN = 1             # first step handled with the stale/fresh split


# ----------------------------------------------------------------------------
# host-side layout builder
# ----------------------------------------------------------------------------

def build_layout(parent_idx, sort=SORT, nsteps=D):
    """Per-step gather index lists + psum-op metadata + permutations.

    For steps d >= SPLIT_MIN the gather lists contain only STALE parents
    (storage rows < bound_{d-1}); fresh parents (the previous depth's nodes)
    are applied on-device as 16 PE matmuls against a per-step 0/1 selection
    matrix A (columns = this step's storage slots, rows = previous depth's
    local rows) times the previous step's vt tile, which is still in SBUF.
    This removes the wb_{d-1} -> gather_d dependency, so each step's gather
    (Q7 desc-gen + DMA) runs during the PREVIOUS step's compute."""
    parent_idx = np.asarray(parent_idx)
    assert parent_idx.shape == (D, N, P)
    pos = np.zeros((D, N), np.int64)   # pos[d][n] = storage slot of node n
    sig = np.zeros((D, N), np.int64)   # sig[d][j] = node stored at slot j
    rowmap = np.zeros(TOTAL, np.int64)
    rowmap[0], rowmap[1] = 0, 1
    steps = []
    packed_cols = []
    afix_parts = []
    col_off = 0
    for d in range(nsteps):
        bound = 2 + d * N
        pv = parent_idx[d].astype(np.int64).copy()   # [N, P]
        pv[(pv < 0) | (pv >= bound)] = 0             # pad/OOB -> zero row
        split = d >= SPLIT_MIN
        split2 = d >= SPLIT_MIN + 1
        pv_stale = pv.copy()
        if split:
            prev_lo = 2 + (d - 1) * N
            fresh = pv >= prev_lo                    # previous depth's rows
            pv_stale[fresh] = 0
        else:
            fresh = None
        if split2:
            prev2_lo = 2 + (d - 2) * N
            recent = (pv >= prev2_lo) & (pv < prev_lo)   # depth d-2 rows
            pv_stale[recent] = 0
        else:
            recent = None
        if sort:
            k = (pv_stale != 0).sum(1)
            order = np.argsort(-k, kind="stable")
        else:
            order = np.arange(N)
        sig[d] = order
        pos[d, order] = np.arange(N)
        rowmap[bound : bound + N] = bound + pos[d]
        pvs = pv_stale[order]                        # [N, P] node-sorted
        if sort:
            ks = (pvs != 0).sum(1)
            comp = np.zeros_like(pvs)
            for j in range(N):
                nz = pvs[j][pvs[j] != 0]
                comp[j, : len(nz)] = nz
            pvs = comp
            cps = [int((ks > p).sum()) for p in range(P)]
        else:
            cps = [N] * P

        def build_A(mask, lo):
            # A[kc*4+mc][q%128, s%128]: parent multiplicity of node at slot s
            # for source local row q (q = rowmap - lo)
            A = np.zeros((16, 128, 128), np.float32)
            nn_, pp_ = np.nonzero(mask)
            for n0, p0 in zip(nn_, pp_):
                q = rowmap[pv[n0, p0]] - lo
                s = pos[d][n0]
                A[(q // 128) * 4 + (s // 128), q % 128, s % 128] += 1.0
            afix_parts.append(
                np.ascontiguousarray(A.transpose(1, 0, 2).reshape(128, 2048)))
            return len(afix_parts) - 1

        fix = build_A(fresh, prev_lo) if split else -1
        fix2 = build_A(recent, prev2_lo) if split2 else -1

        segs = []          # (p, global col offset, padded len)
        idx_parts = []
        off = 0
        for p in range(P):
            cp = cps[p]
            cpp = N if p == 0 else (cdiv(cp, 128) * 128 if cp > 0 else 0)
            if cpp == 0:
                continue
            col = np.zeros(cpp, np.int64)
            m = min(cp, cpp)
            col[:m] = rowmap[pvs[:m, p]]
            idx_parts.append(col)
            segs.append((p, off, cpp))
            off += cpp
        L = off
        idx_list = np.concatenate(idx_parts)
        assert idx_list.shape == (L,) and L % 128 == 0
        assert idx_list.max() < 2 ** 15

        # psum ops: slot segment -> per-G-tile runs (G tiles are CH chunks)
        nch = L // 128
        ntiles = cdiv(nch, CH)
        ops = []   # (is_copy, dst_chunk, nchunks, tile, local_chunk)
        for (p, soff, cpp) in segs:
            gc0 = soff // 128
            ncht = cpp // 128
            c = 0
            while c < ncht:
                t = (gc0 + c) // CH
                lc = (gc0 + c) % CH
                run = min(CH - lc, ncht - c)
                ops.append((p == 0, c, run, t, lc))
                c += run

        # pack idx list: linear i -> partition i%16, col i//16; replicate x8
        cols = L // 16
        pk = idx_list.astype(np.int16).reshape(cols, 16).T
        packed_cols.append(np.tile(pk, (8, 1)))
        steps.append(dict(L=L, nch=nch, ntiles=ntiles, ops=ops,
                          col_off=col_off, cols=cols, bound=bound,
                          split=split, split2=split2, fix=fix, fix2=fix2))
        col_off += cols

    idx_packed = np.concatenate(packed_cols, axis=1)   # [128, col_off]
    afix = (np.stack(afix_parts) if afix_parts
            else np.zeros((1, 128, 2048), np.float32))  # [NFIX, 128, 2048]
    return dict(steps=steps, idx_packed=idx_packed, pos=pos, sig=sig,
                total_cols=col_off, afix=afix)


# ----------------------------------------------------------------------------
# device kernel
# ----------------------------------------------------------------------------

def build_nc(meta, repeat=1):
    nc = bacc.Bacc("TRN2", target_bir_lowering=False, debug=False,
                   dynamic_dma_scratch_size=SCRATCH)

    bufrows = nc.declare_dram_parameter("bufrows", [TOTAL, BL, H], BF16,
                                        isOutput=True)
    init2 = nc.declare_dram_parameter("init2", [2, BL, H], BF16, isOutput=False)
    embt = nc.declare_dram_parameter("embt", [E, D * N], BF16, isOutput=False)
    w1pv_d = nc.declare_dram_parameter("w1pv", [H, H], BF16, isOutput=False)
    w1ne_d = nc.declare_dram_parameter("w1ne", [E, H], BF16, isOutput=False)
    w2t_d = nc.declare_dram_parameter("w2t", [H, H], BF16, isOutput=False)
    b1_d = nc.declare_dram_parameter("b1", [H, 1], F32, isOutput=False)
    b2_d = nc.declare_dram_parameter("b2", [H, 1], F32, isOutput=False)
    ident_d = nc.declare_dram_parameter("ident", [H, H], BF16, isOutput=False)
    idxs_d = nc.declare_dram_parameter("idxs", [128, meta["total_cols"]], I16,
                                       isOutput=False)
    nfix = meta["afix"].shape[0]
    afix_d = nc.declare_dram_parameter("afix", [nfix, 128, 2048], BF16,
                                       isOutput=False)

    steps = meta["steps"]

    with tile.TileContext(nc) as tc, ExitStack() as ctx:
        const = ctx.enter_context(tc.tile_pool(name="const", bufs=1))
        gpool = ctx.enter_context(tc.tile_pool(name="g", bufs=8))
        apool = ctx.enter_context(tc.tile_pool(name="acc", bufs=2))
        spool = ctx.enter_context(tc.tile_pool(name="s", bufs=4))
        nepool = ctx.enter_context(tc.tile_pool(name="ne", bufs=2))
        vpool = ctx.enter_context(tc.tile_pool(name="v", bufs=3))
        afpool = ctx.enter_context(tc.tile_pool(name="af", bufs=2))
        psum = ctx.enter_context(tc.tile_pool(name="ps", bufs=4, space="PSUM"))
        psumf = ctx.enter_context(tc.tile_pool(name="psf", bufs=3, space="PSUM"))

        # constants
        idxs_sb = const.tile([128, meta["total_cols"]], I16, tag="idxs")
        nc.sync.dma_start(idxs_sb[:], idxs_d[:])
        w1pv = const.tile([H, H], BF16, tag="w1pv")
        nc.sync.dma_start(w1pv[:], w1pv_d[:])
        w1ne = const.tile([E, H], BF16, tag="w1ne")
        nc.sync.dma_start(w1ne[:], w1ne_d[:])
        w2t = const.tile([H, H], BF16, tag="w2t")
        nc.sync.dma_start(w2t[:], w2t_d[:])
        b1 = const.tile([H, 1], F32, tag="b1")
        nc.sync.dma_start(b1[:], b1_d[:])
        b2 = const.tile([H, 1], F32, tag="b2")
        nc.sync.dma_start(b2[:], b2_d[:])
        identf = const.tile([H, H], BF16, tag="ident")
        nc.sync.dma_start(identf[:], ident_d[:])

        # init rows 0 (zeros) and 1 (root embedding)
        nc.sync.dma_start(bufrows[0:2, :, :], init2[:])

        def emit_gathers(d):
            s = steps[d]
            # split2 steps read only rows < bound_{d-2}: two full steps of
            # slack to the writeback, so the gather never waits on compute.
            hi = s["bound"]
            if s["split"]:
                hi -= N
            if s["split2"]:
                hi -= N
            src = bufrows[0:hi, :, :].rearrange("r b h -> r (b h)")
            gts = []
            for t in range(s["ntiles"]):
                ncht = min(CH, s["nch"] - t * CH)
                Lt = ncht * 128
                g = gpool.tile([128, CH, BL, H], BF16, tag="g")
                c0 = s["col_off"] + t * CH * 8
                nc.gpsimd.dma_gather(
                    g[:, 0:ncht, :, :].rearrange("p c b h -> p c (b h)"),
                    src, idxs_sb[:, c0:c0 + Lt // 16], Lt, Lt, ROW)
                gts.append(g)
            return gts

        def emit_steps():
            vtb_prev = None
            vtb_prev2 = None
            gts_next = None
            for d in range(len(steps)):
                s = steps[d]
                bound = s["bound"]

                gts = gts_next if gts_next is not None else emit_gathers(d)
                gts_next = None

                ne_t = nepool.tile([E, N], BF16, tag="ne")
                nc.sync.dma_start(ne_t[:], embt[:, d * N:(d + 1) * N])

                # P-sum of stale parents into acc [tok%128, nhi, b, f]
                acc = apool.tile([128, 4, BL, H], BF16, tag="acc")
                for (is_copy, dc, ncg, t, lc) in s["ops"]:
                    dst = acc[:, dc:dc + ncg, :, :]
                    gsrc = gts[t][:, lc:lc + ncg, :, :]
                    if is_copy:
                        nc.vector.tensor_copy(dst, gsrc)
                    else:
                        nc.vector.tensor_add(dst, dst, gsrc)

                if s["split"]:
                    # fresh (depth d-1) and recent (depth d-2) parents are
                    # still in SBUF (vtb_prev / vtb_prev2): acc += A.T @ vtb
                    # via accumulating MMs. The A2 half only needs vt_{d-2},
                    # so it executes during the previous step's tail.
                    af = afpool.tile([128, 16 * 128], BF16, tag="af")
                    nc.sync.dma_start(af[:], afix_d[s["fix"], :, :])
                    if s["split2"]:
                        af2 = afpool.tile([128, 16 * 128], BF16, tag="af2")
                        nc.sync.dma_start(af2[:], afix_d[s["fix2"], :, :])
                    for mc in range(4):
                        pf = psumf.tile([128, BL, H], F32, tag="pf")
                        pfv = pf[:].rearrange("p b h -> p (b h)")
                        first = True
                        if s["split2"]:
                            for kc in range(4):
                                i = (kc * 4 + mc) * 128
                                nc.tensor.matmul(
                                    pfv, af2[:, i:i + 128],
                                    vtb_prev2[:, kc, :, :].rearrange(
                                        "p b h -> p (b h)"),
                                    start=first, stop=False)
                                first = False
                        for kc in range(4):
                            i = (kc * 4 + mc) * 128
                            nc.tensor.matmul(
                                pfv, af[:, i:i + 128],
                                vtb_prev[:, kc, :, :].rearrange(
                                    "p b h -> p (b h)"),
                                start=first, stop=(kc == 3))
                            first = False
                        nc.vector.tensor_add(acc[:, mc, :, :],
                                             acc[:, mc, :, :], pf[:])

                # next step's stale gather: desc-gen + DMA overlap the MLP
                # below (no dep on this step's writeback).
                if d + 1 < len(steps) and steps[d + 1]["split"]:
                    gts_next = emit_gathers(d + 1)

                vt = vpool.tile([128, 4, BL, H], BF16, tag="vt")
                for b in range(BL):
                    ps_tp = psum.tile([128, N], BF16, tag="ps")
                    for nhi in range(4):
                        nc.tensor.transpose(ps_tp[:, nhi * 128:(nhi + 1) * 128],
                                            acc[:, nhi, b, :], identf[:])
                    pvT = spool.tile([128, N], BF16, tag="pvT")
                    nc.scalar.activation(pvT[:], ps_tp[:],
                                         mybir.ActivationFunctionType.Copy)
                    ph1 = psum.tile([128, N], F32, tag="ps")
                    nc.tensor.matmul(ph1[:], w1ne[:], ne_t[:], start=True,
                                     stop=False)
                    nc.tensor.matmul(ph1[:], w1pv[:], pvT[:], start=False,
                                     stop=True)
                    h1 = spool.tile([128, N], BF16, tag="h1")
                    nc.scalar.activation(h1[:], ph1[:],
                                         mybir.ActivationFunctionType.Relu,
                                         bias=b1[:])
                    ph2 = psum.tile([128, N], F32, tag="ps")
                    nc.tensor.matmul(ph2[:], w2t[:], h1[:], start=True, stop=True)
                    h2 = spool.tile([128, N], BF16, tag="h2")
                    nc.scalar.activation(h2[:], ph2[:],
                                         mybir.ActivationFunctionType.Identity,
                                         bias=b2[:])
                    ps_ht = psum.tile([128, 4, H], BF16, tag="ps")
                    for nhi in range(4):
                        nc.tensor.transpose(ps_ht[:, nhi, :],
                                            h2[:, nhi * 128:(nhi + 1) * 128],
                                            identf[:])
                    nc.vector.tensor_add(vt[:, :, b, :], acc[:, :, b, :],
                                         ps_ht[:])

                vtb_prev2 = vtb_prev
                vtb_prev = vt

                rb = bound
                dst = bufrows[rb:rb + N, :, :].rearrange("(c p) b h -> p c b h",
                                                         p=128)
                nc.sync.dma_start(dst, vt[:])


        if repeat > 1:
            with tc.For_i(0, repeat, 1):
                emit_steps()
        else:
            emit_steps()

    nc.compile()
    return nc


# ----------------------------------------------------------------------------
# entry point
# ----------------------------------------------------------------------------

_CACHE = {}


def _get_compiled(parent_idx):
    key = hashlib.sha1(np.asarray(parent_idx).tobytes()).hexdigest()
    if key not in _CACHE:
        meta = build_layout(parent_idx)
        nc = build_nc(meta)
        _CACHE[key] = (nc, meta)
    return _CACHE[key]


def _prepare(inputs):
    embedding = np.asarray(inputs["embedding"], np.float32)
    emb_table = np.asarray(inputs["emb_table"], np.float32)
    W1 = np.asarray(inputs["W1"], np.float32)
    b1 = np.asarray(inputs["b1"], np.float32)
    W2 = np.asarray(inputs["W2"], np.float32)
    b2 = np.asarray(inputs["b2"], np.float32)
    parent_idx = np.asarray(inputs["parent_idx"])

    nc, meta = _get_compiled(parent_idx)
    sig = meta["sig"]

    import ml_dtypes
    bf16 = ml_dtypes.bfloat16
    embt = np.ascontiguousarray(
        np.concatenate([emb_table[2 + d * N + sig[d]] for d in range(D)],
                       axis=0).T)                        # [E, D*N]
    shared = dict(
        embt=embt.astype(bf16),
        w1pv=np.ascontiguousarray(W1[:, :H].T).astype(bf16),
        w1ne=np.ascontiguousarray(W1[:, H:].T).astype(bf16),
        w2t=np.ascontiguousarray(W2.T).astype(bf16),
        b1=b1.reshape(H, 1).copy(),
        b2=b2.reshape(H, 1).copy(),
        ident=np.eye(H, dtype=np.float32).astype(bf16),
        idxs=meta["idx_packed"],
        afix=meta["afix"].astype(bf16),
    )
    in_maps = []
    for c in range(NCORES):
        init2 = np.zeros((2, BL, H), np.float32)
        init2[1] = embedding[c * BL:(c + 1) * BL]
        in_maps.append(dict(shared, init2=init2.astype(bf16)))
    return nc, meta, in_maps


def _run(inputs, trace=False):
    nc, meta, in_maps = _prepare(inputs)
    res = run_bass_kernel_spmd(nc, in_maps, list(range(NCORES)), trace=trace)

    # un-permute rows into the reference output layout
    R = np.empty(TOTAL - 1, np.int64)
    R[0] = 1
    pos = meta["pos"]
    for d in range(D):
        R[1 + d * N: 1 + (d + 1) * N] = 2 + d * N + pos[d]
    out = np.empty((B, TOTAL - 1, H), np.float32)
    for c in range(NCORES):
        br = np.asarray(res.results[c]["bufrows"], np.float32)  # [TOTAL, BL, H]
        out[c * BL:(c + 1) * BL] = br[R].transpose(1, 0, 2)
    return out, res


def kernel(**inputs) -> np.ndarray:
    out, _ = _run(inputs, trace=False)
    return out



# revision 38
# speedup vs baseline: 1.6561x; 1.0354x over previous
"""Trainium2 Bass kernel for nn_DAGModel_88630945120510 (gnn_message_passing).

Data-parallel over batch: 32 batches -> 8 cores x 4 batches. The node buffer
lives in DRAM as [16386 rows, 4 batch, 128] bf16 so one row (1024B) holds all
4 local batches of one node vector -> single gather descriptor. Per depth
step: dma_gather parent rows (host-precomputed p-major index lists) -> DVE
bf16 sum over parent slots -> PE transpose (bf16) -> bf16 MLP (f32 PSUM
accumulate) with ACT relu/bias -> bf16 residual add in token-major ->
contiguous DMA write of the new 512 rows.

Two-depth split: the gather lists contain only STALE parents (rows older
than two depths); fresh (depth d-1) and recent (depth d-2) parents are
applied on-device as accumulating PE matmuls against per-step 0/1 selection
matrices (A, A2) times the vt tiles still in SBUF. This gives the SWDGE
gather two full steps of slack to the writeback (Q7 desc-gen, the critical
resource at ~8.5ns/row, never waits on compute) and removes ~1/d + 1/d of
the gather rows per step. The A2 half only needs vt_{d-2}, so it executes
during the previous step's tail.

Host-side (allowed: plain input marshalling): indices are remapped/sorted
per step (nodes sorted desc by #nonzero stale parents; parents compacted) so
the gather only reads non-padding stale parents. Node order inside each
depth block is permuted; emb_table columns are pre-permuted to match and the
final output is un-permuted (and cast back to f32) on the host.
"""

import hashlib
import numpy as np
from contextlib import ExitStack

import concourse.bass as bass
import concourse.mybir as mybir
import concourse.tile as tile
from concourse import bacc
from concourse.bass_utils import run_bass_kernel_spmd
from concourse._compat import cdiv

F32 = mybir.dt.float32
F32R = mybir.dt.float32r
BF16 = mybir.dt.bfloat16
I16 = mybir.dt.int16

B, H, E = 32, 128, 64
D, N, P = 32, 512, 8
TOTAL = 2 + D * N
NCORES = 8
BL = B // NCORES          # batches per core
ROW = BL * H              # fp32 elems per DRAM row record
CH = 7                    # G-tile chunks; 896 idx/gather
SORT = True               # sorted-prefix gather (skip zero parents)
SCRATCH = 65536           # SWDGE ring bytes/partition (4096 desc slots)
SPLIT_MIN = 1             # first step handled with the stale/fresh split


# ----------------------------------------------------------------------------
# host-side layout builder
# ----------------------------------------------------------------------------

def build_layout(parent_idx, sort=SORT, nsteps=D):
    """Per-step gather index lists + psum-op metadata + permutations.

    For steps d >= SPLIT_MIN the gather lists contain only STALE parents
    (storage rows < bound_{d-1}); fresh parents (the previous depth's nodes)
    are applied on-device as 16 PE matmuls against a per-step 0/1 selection
    matrix A (columns = this step's storage slots, rows = previous depth's
    local rows) times the previous step's vt tile, which is still in SBUF.
    This removes the wb_{d-1} -> gather_d dependency, so each step's gather
    (Q7 desc-gen + DMA) runs during the PREVIOUS step's compute."""
    parent_idx = np.asarray(parent_idx)
    assert parent_idx.shape == (D, N, P)
    pos = np.zeros((D, N), np.int64)   # pos[d][n] = storage slot of node n
    sig = np.zeros((D, N), np.int64)   # sig[d][j] = node stored at slot j
    rowmap = np.zeros(TOTAL, np.int64)
    rowmap[0], rowmap[1] = 0, 1
    steps = []
    packed_cols = []
    afix_parts = []
    col_off = 0
    for d in range(nsteps):
        bound = 2 + d * N
        pv = parent_idx[d].astype(np.int64).copy()   # [N, P]
        pv[(pv < 0) | (pv >= bound)] = 0             # pad/OOB -> zero row
        split = d >= SPLIT_MIN
        split2 = d >= SPLIT_MIN + 1
        pv_stale = pv.copy()
        if split:
            prev_lo = 2 + (d - 1) * N
            fresh = pv >= prev_lo                    # previous depth's rows
            pv_stale[fresh] = 0
        else:
            fresh = None
        if split2:
            prev2_lo = 2 + (d - 2) * N
            recent = (pv >= prev2_lo) & (pv < prev_lo)   # depth d-2 rows
            pv_stale[recent] = 0
        else:
            recent = None
        if sort:
            k = (pv_stale != 0).sum(1)
            order = np.argsort(-k, kind="stable")
        else:
            order = np.arange(N)
        sig[d] = order
        pos[d, order] = np.arange(N)
        rowmap[bound : bound + N] = bound + pos[d]
        pvs = pv_stale[order]                        # [N, P] node-sorted
        if d == 0:
            k0_counts = (pvs != 0).sum(1).astype(np.float32)
        if sort:
            ks = (pvs != 0).sum(1)
            comp = np.zeros_like(pvs)
            for j in range(N):
                nz = pvs[j][pvs[j] != 0]
                comp[j, : len(nz)] = nz
            pvs = comp
            cps = [int((ks > p).sum()) for p in range(P)]
        else:
            cps = [N] * P

        def build_A(mask, lo):
            # A[kc*4+mc][q%128, s%128]: parent multiplicity of node at slot s
            # for source local row q (q = rowmap - lo)
            A = np.zeros((16, 128, 128), np.float32)
            nn_, pp_ = np.nonzero(mask)
            for n0, p0 in zip(nn_, pp_):
                q = rowmap[pv[n0, p0]] - lo
                s = pos[d][n0]
                A[(q // 128) * 4 + (s // 128), q % 128, s % 128] += 1.0
            afix_parts.append(
                np.ascontiguousarray(A.transpose(1, 0, 2).reshape(128, 2048)))
            return len(afix_parts) - 1

        fix = build_A(fresh, prev_lo) if split else -1
        fix2 = build_A(recent, prev2_lo) if split2 else -1

        if d == 0:
            # step 0's parents are all the root row: replace the gather with
            # rank-1 matmuls acc = k0 (x) root on device. k0[s] = parent
            # count of the node stored at slot s.
            cps = [0] * P

        segs = []          # (p, global col offset, padded len)
        idx_parts = []
        off = 0
        for p in range(P):
            cp = cps[p]
            cpp = N if p == 0 else (cdiv(cp, 128) * 128 if cp > 0 else 0)
            if cpp == 0:
                continue
            col = np.zeros(cpp, np.int64)
            m = min(cp, cpp)
            col[:m] = rowmap[pvs[:m, p]]
            idx_parts.append(col)
            segs.append((p, off, cpp))
            off += cpp
        L = off
        idx_list = (np.concatenate(idx_parts) if idx_parts
                    else np.zeros(0, np.int64))
        assert idx_list.shape == (L,) and L % 128 == 0
        assert L == 0 or idx_list.max() < 2 ** 15

        # psum ops: slot segment -> per-G-tile runs (G tiles are CH chunks)
        nch = L // 128
        ntiles = cdiv(nch, CH)
        ops = []   # (is_copy, dst_chunk, nchunks, tile, local_chunk)
        for (p, soff, cpp) in segs:
            gc0 = soff // 128
            ncht = cpp // 128
            c = 0
            while c < ncht:
                t = (gc0 + c) // CH
                lc = (gc0 + c) % CH
                run = min(CH - lc, ncht - c)
                ops.append((p == 0, c, run, t, lc))
                c += run

        # pack idx list: linear i -> partition i%16, col i//16; replicate x8
        cols = L // 16
        pk = idx_list.astype(np.int16).reshape(cols, 16).T
        packed_cols.append(np.tile(pk, (8, 1)))
        steps.append(dict(L=L, nch=nch, ntiles=ntiles, ops=ops,
                          col_off=col_off, cols=cols, bound=bound,
                          split=split, split2=split2, fix=fix, fix2=fix2))
        col_off += cols

    idx_packed = np.concatenate(packed_cols, axis=1)   # [128, col_off]
    afix = (np.stack(afix_parts) if afix_parts
            else np.zeros((1, 128, 2048), np.float32))  # [NFIX, 128, 2048]
    return dict(steps=steps, idx_packed=idx_packed, pos=pos, sig=sig,
                total_cols=col_off, afix=afix, k0=k0_counts.reshape(1, N))


# ----------------------------------------------------------------------------
# device kernel
# ----------------------------------------------------------------------------

def build_nc(meta, repeat=1):
    nc = bacc.Bacc("TRN2", target_bir_lowering=False, debug=False,
                   dynamic_dma_scratch_size=SCRATCH)

    bufrows = nc.declare_dram_parameter("bufrows", [TOTAL, BL, H], BF16,
                                        isOutput=True)
    init2 = nc.declare_dram_parameter("init2", [2, BL, H], BF16, isOutput=False)
    embt = nc.declare_dram_parameter("embt", [E, D * N], BF16, isOutput=False)
    w1pv_d = nc.declare_dram_parameter("w1pv", [H, H], BF16, isOutput=False)
    w1ne_d = nc.declare_dram_parameter("w1ne", [E, H], BF16, isOutput=False)
    w2t_d = nc.declare_dram_parameter("w2t", [H, H], BF16, isOutput=False)
    b1_d = nc.declare_dram_parameter("b1", [H, 1], F32, isOutput=False)
    b2_d = nc.declare_dram_parameter("b2", [H, 1], F32, isOutput=False)
    ident_d = nc.declare_dram_parameter("ident", [H, H], BF16, isOutput=False)
    idxs_d = nc.declare_dram_parameter("idxs", [128, meta["total_cols"]], I16,
                                       isOutput=False)
    k0_d = nc.declare_dram_parameter("k0", [1, N], BF16, isOutput=False)
    nfix = meta["afix"].shape[0]
    afix_d = nc.declare_dram_parameter("afix", [nfix, 128, 2048], BF16,
                                       isOutput=False)

    steps = meta["steps"]

    with tile.TileContext(nc) as tc, ExitStack() as ctx:
        const = ctx.enter_context(tc.tile_pool(name="const", bufs=1))
        gpool = ctx.enter_context(tc.tile_pool(name="g", bufs=10))
        apool = ctx.enter_context(tc.tile_pool(name="acc", bufs=2))
        spool = ctx.enter_context(tc.tile_pool(name="s", bufs=4))
        nepool = ctx.enter_context(tc.tile_pool(name="ne", bufs=2))
        vpool = ctx.enter_context(tc.tile_pool(name="v", bufs=3))
        afpool = ctx.enter_context(tc.tile_pool(name="af", bufs=2))
        psum = ctx.enter_context(tc.tile_pool(name="ps", bufs=4, space="PSUM"))
        psumf = ctx.enter_context(tc.tile_pool(name="psf", bufs=3, space="PSUM"))

        # constants
        idxs_sb = const.tile([128, meta["total_cols"]], I16, tag="idxs")
        nc.sync.dma_start(idxs_sb[:], idxs_d[:])
        w1pv = const.tile([H, H], BF16, tag="w1pv")
        nc.sync.dma_start(w1pv[:], w1pv_d[:])
        w1ne = const.tile([E, H], BF16, tag="w1ne")
        nc.sync.dma_start(w1ne[:], w1ne_d[:])
        w2t = const.tile([H, H], BF16, tag="w2t")
        nc.sync.dma_start(w2t[:], w2t_d[:])
        b1 = const.tile([H, 1], F32, tag="b1")
        nc.sync.dma_start(b1[:], b1_d[:])
        b2 = const.tile([H, 1], F32, tag="b2")
        nc.sync.dma_start(b2[:], b2_d[:])
        identf = const.tile([H, H], BF16, tag="ident")
        nc.sync.dma_start(identf[:], ident_d[:])
        k0t = const.tile([1, N], BF16, tag="k0")
        nc.sync.dma_start(k0t[:], k0_d[:])
        rt = const.tile([1, BL * H], BF16, tag="rt")
        nc.sync.dma_start(rt[:], init2[1:2, :, :].rearrange("a b h -> a (b h)"))

        # init rows 0 (zeros) and 1 (root embedding)
        nc.sync.dma_start(bufrows[0:2, :, :], init2[:])

        def emit_gathers(d):
            s = steps[d]
            # split2 steps read only rows < bound_{d-2}: two full steps of
            # slack to the writeback, so the gather never waits on compute.
            hi = s["bound"]
            if s["split"]:
                hi -= N
            if s["split2"]:
                hi -= N
            src = bufrows[0:hi, :, :].rearrange("r b h -> r (b h)")
            gts = []
            for t in range(s["ntiles"]):
                ncht = min(CH, s["nch"] - t * CH)
                Lt = ncht * 128
                g = gpool.tile([128, CH, BL, H], BF16, tag="g")
                c0 = s["col_off"] + t * CH * 8
                nc.gpsimd.dma_gather(
                    g[:, 0:ncht, :, :].rearrange("p c b h -> p c (b h)"),
                    src, idxs_sb[:, c0:c0 + Lt // 16], Lt, Lt, ROW)
                gts.append(g)
            return gts

        def emit_steps():
            vtb_prev = None
            vtb_prev2 = None
            gts_next = None
            for d in range(len(steps)):
                s = steps[d]
                bound = s["bound"]

                gts = gts_next if gts_next is not None else emit_gathers(d)
                gts_next = None

                ne_t = nepool.tile([E, N], BF16, tag="ne")
                nc.sync.dma_start(ne_t[:], embt[:, d * N:(d + 1) * N])

                # P-sum of stale parents into acc [tok%128, nhi, b, f]
                acc = apool.tile([128, 4, BL, H], BF16, tag="acc")
                for (is_copy, dc, ncg, t, lc) in s["ops"]:
                    dst = acc[:, dc:dc + ncg, :, :]
                    gsrc = gts[t][:, lc:lc + ncg, :, :]
                    if is_copy:
                        nc.vector.tensor_copy(dst, gsrc)
                    else:
                        nc.vector.tensor_add(dst, dst, gsrc)
                if d == 0:
                    # acc[s, b, h] = k0[s] * root[b, h] via rank-1 matmuls
                    for mc in range(4):
                        pf = psumf.tile([128, BL, H], F32, tag="pf")
                        nc.tensor.matmul(
                            pf[:].rearrange("p b h -> p (b h)"),
                            k0t[0:1, mc * 128:(mc + 1) * 128], rt[0:1, :],
                            start=True, stop=True)
                        nc.scalar.activation(
                            acc[:, mc, :, :], pf[:],
                            mybir.ActivationFunctionType.Copy)

                if s["split"]:
                    # fresh (depth d-1) and recent (depth d-2) parents are
                    # still in SBUF (vtb_prev / vtb_prev2): acc += A.T @ vtb
                    # via accumulating MMs. The A2 half only needs vt_{d-2},
                    # so it executes during the previous step's tail.
                    af = afpool.tile([128, 16 * 128], BF16, tag="af")
                    nc.sync.dma_start(af[:], afix_d[s["fix"], :, :])
                    if s["split2"]:
                        af2 = afpool.tile([128, 16 * 128], BF16, tag="af2")
                        nc.sync.dma_start(af2[:], afix_d[s["fix2"], :, :])
                    for mc in range(4):
                        pf = psumf.tile([128, BL, H], F32, tag="pf")
                        pfv = pf[:].rearrange("p b h -> p (b h)")
                        first = True
                        if s["split2"]:
                            for kc in range(4):
                                i = (kc * 4 + mc) * 128
                                nc.tensor.matmul(
                                    pfv, af2[:, i:i + 128],
                                    vtb_prev2[:, kc, :, :].rearrange(
                                        "p b h -> p (b h)"),
                                    start=first, stop=False)
                                first = False
                        for kc in range(4):
                            i = (kc * 4 + mc) * 128
                            nc.tensor.matmul(
                                pfv, af[:, i:i + 128],
                                vtb_prev[:, kc, :, :].rearrange(
                                    "p b h -> p (b h)"),
                                start=first, stop=(kc == 3))
                            first = False
                        nc.vector.tensor_add(acc[:, mc, :, :],
                                             acc[:, mc, :, :], pf[:])

                # next step's stale gather: desc-gen + DMA overlap the MLP
                # below (no dep on this step's writeback).
                if d + 1 < len(steps) and steps[d + 1]["split"]:
                    gts_next = emit_gathers(d + 1)

                vt = vpool.tile([128, 4, BL, H], BF16, tag="vt")
                for b in range(BL):
                    ps_tp = psum.tile([128, N], BF16, tag="ps")
                    for nhi in range(4):
                        nc.tensor.transpose(ps_tp[:, nhi * 128:(nhi + 1) * 128],
                                            acc[:, nhi, b, :], identf[:])
                    pvT = spool.tile([128, N], BF16, tag="pvT")
                    nc.scalar.activation(pvT[:], ps_tp[:],
                                         mybir.ActivationFunctionType.Copy)
                    ph1 = psum.tile([128, N], F32, tag="ps")
                    nc.tensor.matmul(ph1[:], w1ne[:], ne_t[:], start=True,
                                     stop=False)
                    nc.tensor.matmul(ph1[:], w1pv[:], pvT[:], start=False,
                                     stop=True)
                    h1 = spool.tile([128, N], BF16, tag="h1")
                    nc.scalar.activation(h1[:], ph1[:],
                                         mybir.ActivationFunctionType.Relu,
                                         bias=b1[:])
                    ph2 = psum.tile([128, N], F32, tag="ps")
                    nc.tensor.matmul(ph2[:], w2t[:], h1[:], start=True, stop=True)
                    h2 = spool.tile([128, N], BF16, tag="h2")
                    nc.scalar.activation(h2[:], ph2[:],
                                         mybir.ActivationFunctionType.Identity,
                                         bias=b2[:])
                    ps_ht = psum.tile([128, 4, H], BF16, tag="ps")
                    for nhi in range(4):
                        nc.tensor.transpose(ps_ht[:, nhi, :],
                                            h2[:, nhi * 128:(nhi + 1) * 128],
                                            identf[:])
                    nc.vector.tensor_add(vt[:, :, b, :], acc[:, :, b, :],
                                         ps_ht[:])

                vtb_prev2 = vtb_prev
                vtb_prev = vt

                rb = bound
                dst = bufrows[rb:rb + N, :, :].rearrange("(c p) b h -> p c b h",
                                                         p=128)
                nc.sync.dma_start(dst, vt[:])


        if repeat > 1:
            with tc.For_i(0, repeat, 1):
                emit_steps()
        else:
            emit_steps()

    nc.compile()
    return nc


# ----------------------------------------------------------------------------
# entry point
# ----------------------------------------------------------------------------

_CACHE = {}


def _get_compiled(parent_idx):
    key = hashlib.sha1(np.asarray(parent_idx).tobytes()).hexdigest()
    if key not in _CACHE:
        meta = build_layout(parent_idx)
        nc = build_nc(meta)
        _CACHE[key] = (nc, meta)
    return _CACHE[key]


def _prepare(inputs):
    embedding = np.asarray(inputs["embedding"], np.float32)
    emb_table = np.asarray(inputs["emb_table"], np.float32)
    W1 = np.asarray(inputs["W1"], np.float32)
    b1 = np.asarray(inputs["b1"], np.float32)
    W2 = np.asarray(inputs["W2"], np.float32)
    b2 = np.asarray(inputs["b2"], np.float32)
    parent_idx = np.asarray(inputs["parent_idx"])

    nc, meta = _get_compiled(parent_idx)
    sig = meta["sig"]

    import ml_dtypes
    bf16 = ml_dtypes.bfloat16
    embt = np.ascontiguousarray(
        np.concatenate([emb_table[2 + d * N + sig[d]] for d in range(D)],
                       axis=0).T)                        # [E, D*N]
    shared = dict(
        embt=embt.astype(bf16),
        w1pv=np.ascontiguousarray(W1[:, :H].T).astype(bf16),
        w1ne=np.ascontiguousarray(W1[:, H:].T).astype(bf16),
        w2t=np.ascontiguousarray(W2.T).astype(bf16),
        b1=b1.reshape(H, 1).copy(),
        b2=b2.reshape(H, 1).copy(),
        ident=np.eye(H, dtype=np.float32).astype(bf16),
        idxs=meta["idx_packed"],
        afix=meta["afix"].astype(bf16),
        k0=meta["k0"].astype(bf16),
    )
    in_maps = []
    for c in range(NCORES):
        init2 = np.zeros((2, BL, H), np.float32)
        init2[1] = embedding[c * BL:(c + 1) * BL]
        in_maps.append(dict(shared, init2=init2.astype(bf16)))
    return nc, meta, in_maps


def _run(inputs, trace=False):
    nc, meta, in_maps = _prepare(inputs)
    res = run_bass_kernel_spmd(nc, in_maps, list(range(NCORES)), trace=trace)

    # un-permute rows into the reference output layout
    R = np.empty(TOTAL - 1, np.int64)
    R[0] = 1
    pos = meta["pos"]
    for d in range(D):
        R[1 + d * N: 1 + (d + 1) * N] = 2 + d * N + pos[d]
    out = np.empty((B, TOTAL - 1, H), np.float32)
    for c in range(NCORES):
        br = np.asarray(res.results[c]["bufrows"], np.float32)  # [TOTAL, BL, H]
        out[c * BL:(c + 1) * BL] = br[R].transpose(1, 0, 2)
    return out, res


def kernel(**inputs) -> np.ndarray:
    out, _ = _run(inputs, trace=False)
    return out

